# revision 7
# baseline (speedup 1.0000x reference)
"""Tensor-parallel attention kernel for 8 TRN2 NeuronCores.

Sharding (Megatron-style over heads):
  - core c owns heads 2c, 2c+1: qkv_weight rows [c*768,(c+1)*768), qkv_bias same.
  - QKV + causal attention computed per-core in bf16 (f32 PSUM accum).
  - context (T-layout [dim, tok]) AllGather'd across cores -> every core has the
    full [2048, tok] context.
  - dense out-dims sharded: core c computes out[:, c*256:(c+1)*256) using
    dense_weight rows [c*256,(c+1)*256); host concatenates along dim 1.

Softmax: no max-subtraction (scores are O(5), exp safe in f32); causal mask
applied post-exp via affine_select fill=0; fully-masked tiles skipped.
"""

import math
import os

import numpy as np

import concourse.bass as bass
import concourse.mybir as mybir
import concourse.tile as tile
from concourse import bacc
from concourse.bass_utils import run_bass_kernel_spmd

F32 = mybir.dt.float32
BF16 = mybir.dt.bfloat16

SEQ = 2048
BATCH = 2
HID = 2048
NH = 16
HD = 128
NCORES = 8
HPC = NH // NCORES            # heads per core = 2
QROWS = 3 * HD * HPC          # qkv rows per core = 768
DPC = HD * HPC                # context dims per core = 256
TOK = SEQ * BATCH             # 4096 flat tokens, tok = s*BATCH + b
NPANEL = 8
PTOK = TOK // NPANEL          # 512 tokens per panel (= 256 s per batch)
PS = PTOK // BATCH            # 256 s-rows per panel per batch
KC = HID // 128               # 16 contraction chunks
SB = 512                      # attention s-block (queries per block, per batch)
NSB = SEQ // SB               # 4 s-blocks
SCALE = 1.0 / math.sqrt(HD)

_compiled = None


def _build():
    nc = bacc.Bacc(
        "TRN2",
        target_bir_lowering=False,
        debug=False,
        enable_asserts=False,
        num_devices=NCORES,
    )

    hidden = nc.declare_dram_parameter("hidden", [TOK, HID], F32, isOutput=False)
    wqkv = nc.declare_dram_parameter("wqkv", [QROWS, HID], F32, isOutput=False)
    bqkv = nc.declare_dram_parameter("bqkv", [1, QROWS], F32, isOutput=False)
    wd = nc.declare_dram_parameter("wd", [DPC, HID], F32, isOutput=False)
    out = nc.declare_dram_parameter("out", [TOK, DPC], F32, isOutput=True)

    ag_in = nc.dram_tensor("ag_in", [DPC, BATCH, SEQ], BF16)
    ag_out = nc.dram_tensor("ag_out", [HID, BATCH, SEQ], BF16, addr_space="Shared")

    rg = [list(range(NCORES))]

    with tile.TileContext(nc) as tc:
        with (
            tc.tile_pool(name="persist", bufs=1) as pers,
            tc.tile_pool(name="dram", bufs=3, space="DRAM") as dram,
        ):
            # ---- persistent SBUF tensors ----
            wqT = pers.tile([128, KC, QROWS], BF16, tag="wqT")       # qkv W^T
            wdT = pers.tile([128, KC, DPC], BF16, tag="wdT")         # dense W^T slice
            qT = pers.tile([128, HPC, BATCH, SEQ], BF16, tag="qT")   # [dv,h,b,s]
            kT = pers.tile([128, HPC, BATCH, SEQ], BF16, tag="kT")
            vst = pers.tile([128, BATCH, HPC, SEQ // 128, HD], BF16, tag="vst")
            bias_sb = pers.tile([1, QROWS], BF16, tag="bias")
            ones_bf = pers.tile([1, SB], BF16, tag="ones_bf")
            ones_col = pers.tile([128, 1], F32, tag="ones_col")
            ones_row = pers.tile([1, 128], F32, tag="ones_row")

            nc.gpsimd.memset(ones_bf[:], 1.0)
            nc.gpsimd.memset(ones_col[:], 1.0)
            nc.gpsimd.memset(ones_row[:], 1.0)

            # ---- weight prep: cast to bf16 in DRAM, transpose-load to SBUF ----
            with tc.tile_pool(name="wprep", bufs=3) as wp:
                bias_f32 = wp.tile([1, QROWS], F32, tag="bias_f32")
                nc.sync.dma_start(bias_f32[:], bqkv[:])
                nc.vector.tensor_copy(bias_sb[:], bias_f32[:])

                wq16 = dram.tile([QROWS, HID], BF16, tag="wq16")
                for i in range(QROWS // 128):
                    t32 = wp.tile([128, HID], F32, tag="w32")
                    t16 = wp.tile([128, HID], BF16, tag="w16")
                    nc.sync.dma_start(t32[:], wqkv[i * 128 : (i + 1) * 128, :])
                    nc.vector.tensor_copy(t16[:], t32[:])
                    nc.sync.dma_start(wq16[i * 128 : (i + 1) * 128, :], t16[:])
                wd16 = dram.tile([DPC, HID], BF16, tag="wd16")
                for i in range(DPC // 128):
                    t32 = wp.tile([128, HID], F32, tag="w32")
                    t16 = wp.tile([128, HID], BF16, tag="w16")
                    nc.sync.dma_start(t32[:], wd[i * 128 : (i + 1) * 128, :])
                    nc.vector.tensor_copy(t16[:], t32[:])
                    nc.sync.dma_start(wd16[i * 128 : (i + 1) * 128, :], t16[:])
                for kc in range(KC):
                    nc.sync.dma_start_transpose(
                        wqT[:, kc, :], wq16[:, kc * 128 : (kc + 1) * 128]
                    )
                    nc.sync.dma_start_transpose(
                        wdT[:, kc, :], wd16[:, kc * 128 : (kc + 1) * 128]
                    )

            # ---- phase 1: hidden cast + transpose + QKV projection ----
            with (
                tc.tile_pool(name="hload", bufs=3) as hl,
                tc.tile_pool(name="hidT", bufs=2) as hp,
                tc.tile_pool(name="qkvps", bufs=2, space="PSUM") as qkps,
                tc.tile_pool(name="vps", bufs=2, space="PSUM") as vps,
            ):
                for p in range(NPANEL):
                    h16 = dram.tile([PTOK, HID], BF16, tag="h16")
                    for tt in range(PTOK // 128):
                        h32 = hl.tile([128, HID], F32, tag="h32")
                        ht = hl.tile([128, HID], BF16, tag="ht")
                        nc.sync.dma_start(
                            h32[:], hidden[p * PTOK + tt * 128 : p * PTOK + (tt + 1) * 128, :]
                        )
                        # split cast work between DVE and ACT
                        if tt % 2 == 0:
                            nc.vector.tensor_copy(ht[:], h32[:])
                        else:
                            nc.scalar.copy(ht[:], h32[:])
                        nc.sync.dma_start(h16[tt * 128 : (tt + 1) * 128, :], ht[:])
                    hidT = hp.tile([128, KC, PTOK], BF16, tag="hidT")
                    for kc in range(KC):
                        nc.sync.dma_start_transpose(
                            hidT[:, kc, :], h16[:, kc * 128 : (kc + 1) * 128]
                        )
                    # q/k rows (T layout): row-blocks m: 0=q.h0 1=k.h0 2=v.h0 3=q.h1 ...
                    for m in (0, 1, 3, 4):
                        h = m // 3
                        dst = qT if (m % 3 == 0) else kT
                        ps = qkps.tile([128, PTOK], F32, tag="qk")
                        nc.tensor.matmul(
                            ps[:],
                            bias_sb[:, m * 128 : (m + 1) * 128],
                            ones_bf[:, :PTOK],
                            start=True,
                            stop=False,
                        )
                        for kc in range(KC):
                            nc.tensor.matmul(
                                ps[:],
                                wqT[:, kc, m * 128 : (m + 1) * 128],
                                hidT[:, kc, :],
                                start=False,
                                stop=(kc == KC - 1),
                            )
                        nc.scalar.copy(
                            dst[:, h, :, p * PS : (p + 1) * PS],
                            ps.rearrange("p (s b) -> p b s", b=BATCH),
                        )
                    # v rows (natural layout [t, (h dv)]), per batch, consecutive s
                    vrows = wqT[:, :, :].rearrange(
                        "p k (h three d) -> p k three h d", three=3, d=128
                    )
                    brows = bias_sb.rearrange(
                        "o (h three d) -> o three h d", three=3, d=128
                    )
                    for b in range(BATCH):
                        for tb in range(PS // 128):
                            tc_idx = p * (PS // 128) + tb
                            ps = vps.tile([128, HPC * HD], F32, tag="v")
                            nc.tensor.matmul(
                                ps[:],
                                ones_bf[:, :128],
                                brows[:, 2, :, :],
                                start=True,
                                stop=False,
                            )
                            for kc in range(KC):
                                lhs = hidT[:, kc, :].rearrange(
                                    "p (s b) -> p b s", b=BATCH
                                )[:, b, tb * 128 : (tb + 1) * 128]
                                nc.tensor.matmul(
                                    ps[:],
                                    lhs,
                                    vrows[:, kc, 2, :, :],
                                    start=False,
                                    stop=(kc == KC - 1),
                                )
                            nc.vector.tensor_copy(
                                vst[:, b, :, tc_idx, :],
                                ps.rearrange("p (h d) -> p h d", h=HPC),
                            )

            # ---- phase 2: causal attention per (b, head, s-block) ----
            with (
                tc.tile_pool(name="attn", bufs=4) as ap,
                tc.tile_pool(name="acc", bufs=2) as accp,
                tc.tile_pool(name="sps", bufs=2, space="PSUM") as sps,
                tc.tile_pool(name="cps", bufs=2, space="PSUM") as cps,
                tc.tile_pool(name="sumps", bufs=1, space="PSUM") as sumps,
                tc.tile_pool(name="bps", bufs=1, space="PSUM") as bps,
            ):
                for b in range(BATCH):
                    for h in range(HPC):
                        for sb in range(NSB):
                            s0 = sb * SB
                            q_rhs = qT[:, h, b, s0 : s0 + SB]
                            nchunk = (sb + 1) * (SB // 128)
                            ctx_ps = cps.tile([128, SB], F32, tag="ctx")
                            acc = accp.tile([128, SB], F32, tag="acc")
                            for kc in range(nchunk):
                                s_ps = sps.tile([128, SB], F32, tag="s")
                                nc.tensor.matmul(
                                    s_ps[:],
                                    kT[:, h, b, kc * 128 : (kc + 1) * 128],
                                    q_rhs,
                                    start=True,
                                    stop=True,
                                )
                                e = ap.tile([128, SB], BF16, tag="e")
                                nc.scalar.activation(
                                    e[:],
                                    s_ps[:],
                                    mybir.ActivationFunctionType.Exp,
                                    scale=SCALE,
                                )
                                off = kc * 128 - s0
                                if off >= 0:
                                    # diagonal tile: keep where s >= t, else 0
                                    nc.gpsimd.affine_select(
                                        out=e[:],
                                        in_=e[:],
                                        compare_op=mybir.AluOpType.is_ge,
                                        fill=0.0,
                                        base=-off,
                                        pattern=[[1, SB]],
                                        channel_multiplier=-1,
                                    )
                                if kc == 0:
                                    nc.vector.tensor_copy(acc[:], e[:])
                                else:
                                    nc.vector.tensor_add(acc[:], acc[:], e[:])
                                nc.tensor.matmul(
                                    ctx_ps[:],
                                    vst[:, b, h, kc, :],
                                    e[:],
                                    start=(kc == 0),
                                    stop=(kc == nchunk - 1),
                                )
                            sum_ps = sumps.tile([1, SB], F32, tag="sum")
                            nc.tensor.matmul(
                                sum_ps[:], ones_col[:], acc[:], start=True, stop=True
                            )
                            recip = ap.tile([1, SB], F32, tag="recip")
                            nc.vector.reciprocal(recip[:], sum_ps[:])
                            b_ps = bps.tile([128, SB], F32, tag="bcast")
                            nc.tensor.matmul(
                                b_ps[:], ones_row[:], recip[:], start=True, stop=True
                            )
                            bcast = ap.tile([128, SB], F32, tag="bcsb")
                            nc.scalar.copy(bcast[:], b_ps[:])
                            ctx_sb = ap.tile([128, SB], BF16, tag="ctxsb")
                            nc.vector.tensor_mul(ctx_sb[:], ctx_ps[:], bcast[:])
                            nc.sync.dma_start(
                                ag_in[h * 128 : (h + 1) * 128, b, s0 : s0 + SB],
                                ctx_sb[:],
                            )

            # ---- phase 3: AllGather context + dense (out-dim sharded) ----
            nc.gpsimd.collective_compute(
                "AllGather",
                mybir.AluOpType.bypass,
                replica_groups=rg,
                ins=[ag_in[:].opt()],
                outs=[ag_out[:].opt()],
            )
            with (
                tc.tile_pool(name="gt", bufs=3) as gp,
                tc.tile_pool(name="osb", bufs=4) as op,
                tc.tile_pool(name="dps", bufs=8, space="PSUM") as dps,
            ):
                for b in range(BATCH):
                    for sb in range(NSB):
                        s0 = sb * SB
                        psums = []
                        for _pi in range(SB // 128):
                            dpt = dps.tile([128, DPC], F32, tag="d", name=f"d{_pi}")
                            psums.append(dpt)
                        for kc in range(KC):
                            gt = gp.tile([128, SB], BF16, tag="gt")
                            nc.sync.dma_start(
                                gt[:], ag_out[kc * 128 : (kc + 1) * 128, b, s0 : s0 + SB]
                            )
                            for sub in range(SB // 128):
                                nc.tensor.matmul(
                                    psums[sub][:],
                                    gt[:, sub * 128 : (sub + 1) * 128],
                                    wdT[:, kc, :],
                                    start=(kc == 0),
                                    stop=(kc == KC - 1),
                                )
                        for sub in range(SB // 128):
                            o = op.tile([128, DPC], F32, tag="o")
                            nc.scalar.copy(o[:], psums[sub][:])
                            tok0 = (s0 + sub * 128) * BATCH + b
                            nc.sync.dma_start(
                                out[:].rearrange("(s b) d -> s b d", b=BATCH)[
                                    s0 + sub * 128 : s0 + (sub + 1) * 128, b, :
                                ],
                                o[:],
                            )
    nc.finalize()
    return nc


def _get_nc():
    global _compiled
    if _compiled is None:
        _compiled = _build()
    return _compiled


LAST_RESULTS = {}


def kernel(hidden_states, attention_mask, qkv_weight, qkv_bias, dense_weight, dense_bias):
    nc = _get_nc()
    hs = np.ascontiguousarray(np.asarray(hidden_states, np.float32).reshape(TOK, HID))
    wq = np.asarray(qkv_weight, np.float32)
    bq = np.asarray(qkv_bias, np.float32)
    wde = np.asarray(dense_weight, np.float32)
    in_maps = []
    for c in range(NCORES):
        in_maps.append(
            {
                "hidden": hs,
                "wqkv": np.ascontiguousarray(wq[c * QROWS : (c + 1) * QROWS]),
                "bqkv": np.ascontiguousarray(bq[c * QROWS : (c + 1) * QROWS]).reshape(1, QROWS),
                "wd": np.ascontiguousarray(wde[c * DPC : (c + 1) * DPC]),
            }
        )
    res = run_bass_kernel_spmd(nc, in_maps, core_ids=list(range(NCORES)), trace=False)
    LAST_RESULTS["exec_time_ns"] = res.exec_time_ns
    LAST_RESULTS["res"] = res
    outs = [np.asarray(res.results[c]["out"]) for c in range(NCORES)]
    full = np.concatenate(outs, axis=1).reshape(SEQ, BATCH, HID)
    return full, np.asarray(dense_bias)


def _make_in_maps(hidden_states, qkv_weight, qkv_bias, dense_weight):
    hs = np.ascontiguousarray(np.asarray(hidden_states, np.float32).reshape(TOK, HID))
    wq = np.asarray(qkv_weight, np.float32)
    bq = np.asarray(qkv_bias, np.float32)
    wde = np.asarray(dense_weight, np.float32)
    in_maps = []
    for c in range(NCORES):
        in_maps.append(
            {
                "hidden": hs,
                "wqkv": np.ascontiguousarray(wq[c * QROWS : (c + 1) * QROWS]),
                "bqkv": np.ascontiguousarray(bq[c * QROWS : (c + 1) * QROWS]).reshape(1, QROWS),
                "wd": np.ascontiguousarray(wde[c * DPC : (c + 1) * DPC]),
            }
        )
    return in_maps


def bench_kernel(inputs, iters=8):
    """Steady-state timing: compile the sharded callable once, run `iters`
    times with device-resident inputs, return list of per-call wall seconds."""
    import time

    import jax
    from jax.experimental.shard_map import shard_map
    from jax.sharding import Mesh, PartitionSpec

    from concourse import bass2jax, mybir as _mybir

    nc = _get_nc()
    in_maps = _make_in_maps(
        inputs["hidden_states"], inputs["qkv_weight"], inputs["qkv_bias"],
        inputs["dense_weight"],
    )
    bass2jax.install_neuronx_cc_hook()

    partition_name = nc.partition_id_tensor.name if nc.partition_id_tensor else None
    in_names, out_names, out_avals, zero_outs = [], [], [], []
    for alloc in nc.m.functions[0].allocations:
        if not isinstance(alloc, _mybir.MemoryLocationSet):
            continue
        name = alloc.memorylocations[0].name
        if alloc.kind == "ExternalInput":
            if name != partition_name:
                in_names.append(name)
        elif alloc.kind == "ExternalOutput":
            out_names.append(name)
            np_dtype = _mybir.dt.np(alloc.dtype)
            out_avals.append(jax.core.ShapedArray(tuple(alloc.tensor_shape), np_dtype))
            zero_outs.append(np.zeros(tuple(alloc.tensor_shape), np_dtype))
    n_params = len(in_names)
    all_names = in_names + out_names
    if partition_name is not None:
        all_names = all_names + [partition_name]

    def _body(*args):
        operands = list(args)
        if partition_name is not None:
            operands.append(bass2jax.partition_id_tensor())
        outs = bass2jax._bass_exec_p.bind(
            *operands,
            out_avals=tuple(out_avals),
            in_names=tuple(all_names),
            out_names=tuple(out_names),
            lowering_input_output_aliases=(),
            sim_require_finite=True,
            sim_require_nnan=True,
            nc=nc,
        )
        return tuple(outs)

    devices = jax.devices()[:NCORES]
    mesh = Mesh(np.asarray(devices), ("core",))
    nin = n_params + len(out_names)
    sharded = jax.jit(
        shard_map(
            _body,
            mesh=mesh,
            in_specs=(PartitionSpec("core"),) * nin,
            out_specs=(PartitionSpec("core"),) * len(out_names),
            check_rep=False,
        ),
        keep_unused=True,
    )
    concat_in = [
        np.concatenate([np.asarray(in_maps[c][nm]) for c in range(NCORES)], axis=0)
        for nm in in_names
    ]
    concat_zero = [np.zeros((NCORES * z.shape[0], *z.shape[1:]), z.dtype) for z in zero_outs]
    args = [jax.device_put(a) for a in concat_in + concat_zero]
    out = sharded(*args)  # compile + warmup
    jax.block_until_ready(out)
    times = []
    for _ in range(iters):
        t0 = time.perf_counter()
        out = sharded(*args)
        jax.block_until_ready(out)
        times.append(time.perf_counter() - t0)
    return times, out, out_names


# revision 22
# speedup vs baseline: 1.0049x; 1.0049x over previous
"""Tensor-parallel attention kernel for 8 TRN2 NeuronCores.

Sharding (Megatron-style over heads):
  - core c owns heads 2c, 2c+1: qkv_weight rows [c*768,(c+1)*768), qkv_bias same.
  - QKV + causal attention computed per-core in bf16 (f32 PSUM accum).
  - context (T-layout [dim, tok]) AllGather'd across cores -> every core has the
    full [2048, tok] context.
  - dense out-dims sharded: core c computes out[:, c*256:(c+1)*256) using
    dense_weight rows [c*256,(c+1)*256); host concatenates along dim 1.

Softmax: no max-subtraction (scores are O(5), exp safe in f32); causal mask
applied post-exp via affine_select fill=0; fully-masked tiles skipped.
"""

import math
import os

import numpy as np

import concourse.bass as bass
import concourse.mybir as mybir
import concourse.tile as tile
from concourse import bacc
from concourse.bass_utils import run_bass_kernel_spmd

F32 = mybir.dt.float32
BF16 = mybir.dt.bfloat16

SEQ = 2048
BATCH = 2
HID = 2048
NH = 16
HD = 128
NCORES = 8
HPC = NH // NCORES            # heads per core = 2
QROWS = 3 * HD * HPC          # qkv rows per core = 768
DPC = HD * HPC                # context dims per core = 256
TOK = SEQ * BATCH             # 4096 flat tokens, tok = s*BATCH + b
NPANEL = 8
PTOK = TOK // NPANEL          # 512 tokens per panel (= 256 s per batch)
PS = PTOK // BATCH            # 256 s-rows per panel per batch
TPC = TOK // NCORES           # 512 tokens cast per core (hidden input slice)
KC = HID // 128               # 16 contraction chunks
SB = 512                      # attention s-block (queries per block, per batch)
NSB = SEQ // SB               # 4 s-blocks
SCALE = 1.0 / math.sqrt(HD)

_compiled = None


def _build(sim_mode=False):
    nc = bacc.Bacc(
        "TRN2",
        target_bir_lowering=False,
        debug=False,
        enable_asserts=False,
        num_devices=1 if sim_mode else NCORES,
    )

    hidden = nc.declare_dram_parameter("hidden", [TPC, HID], F32, isOutput=False)
    wqkv = nc.declare_dram_parameter("wqkv", [QROWS, HID], F32, isOutput=False)
    bqkv = nc.declare_dram_parameter("bqkv", [1, QROWS], F32, isOutput=False)
    wd = nc.declare_dram_parameter("wd", [DPC, HID], F32, isOutput=False)
    out = nc.declare_dram_parameter("out", [TOK, DPC], F32, isOutput=True)

    HSEQ = SEQ // 2
    ag_in_a = nc.dram_tensor("ag_in_a", [DPC, BATCH, HSEQ], BF16)
    ag_in_b = nc.dram_tensor("ag_in_b", [DPC, BATCH, HSEQ], BF16)
    ag_out_a = nc.dram_tensor("ag_out_a", [HID, BATCH, HSEQ], BF16, addr_space="Shared")
    ag_out_b = nc.dram_tensor("ag_out_b", [HID, BATCH, HSEQ], BF16, addr_space="Shared")
    h16own = nc.dram_tensor("h16own", [TPC, HID], BF16)
    h16all = nc.dram_tensor("h16all", [TOK, HID], BF16, addr_space="Shared")

    rg = [list(range(NCORES))]

    def all_gather(ag_in, ag_out):
        if sim_mode:
            # stand-in: replicate input into all rank slices so the timeline
            # sees an equivalent-size transfer + the same dependency shape
            sz = ag_out.shape[0] // NCORES
            for c in range(NCORES):
                nc.gpsimd.dma_start(ag_out[c * sz : (c + 1) * sz], ag_in[:])
        else:
            nc.gpsimd.collective_compute(
                "AllGather",
                mybir.AluOpType.bypass,
                replica_groups=rg,
                ins=[ag_in[:].opt()],
                outs=[ag_out[:].opt()],
            )

    with tile.TileContext(nc) as tc:
        with (
            tc.tile_pool(name="persist", bufs=1) as pers,
            tc.tile_pool(name="dram", bufs=3, space="DRAM") as dram,
        ):
            # ---- persistent SBUF tensors ----
            wqT = pers.tile([128, KC, QROWS], BF16, tag="wqT")       # qkv W^T
            wdT = pers.tile([128, KC, DPC], BF16, tag="wdT")         # dense W^T slice
            qT = pers.tile([128, HPC, BATCH, SEQ], BF16, tag="qT")   # [dv,h,b,s]
            kT = pers.tile([128, HPC, BATCH, SEQ], BF16, tag="kT")
            vst = pers.tile([128, BATCH, HPC, SEQ // 128, HD], BF16, tag="vst")
            bias_sb = pers.tile([1, QROWS], BF16, tag="bias")
            ones_bf = pers.tile([1, SB], BF16, tag="ones_bf")
            ones_col = pers.tile([128, 1], F32, tag="ones_col")
            ones_row = pers.tile([1, 128], F32, tag="ones_row")

            nc.gpsimd.memset(ones_bf[:], 1.0)
            nc.gpsimd.memset(ones_col[:], 1.0)
            nc.gpsimd.memset(ones_row[:], 1.0)

            # ---- weight prep (gpsimd DMAs + ACT transposes, off the sync queue) ----
            with tc.tile_pool(name="wprep", bufs=2) as wp:
                bias_f32 = wp.tile([1, QROWS], F32, tag="bias_f32")
                nc.gpsimd.dma_start(bias_f32[:], bqkv[:])
                nc.vector.tensor_copy(bias_sb[:], bias_f32[:])

                wq16 = dram.tile([QROWS, HID], BF16, tag="wq16")
                for i in range(QROWS // 256):
                    t32 = wp.tile([128, 2, HID], F32, tag="w32")
                    t16 = wp.tile([128, 2, HID], BF16, tag="w16")
                    src = wqkv[i * 256 : (i + 1) * 256, :].rearrange(
                        "(a p) h -> p a h", p=128
                    )
                    nc.gpsimd.dma_start(t32[:], src)
                    nc.vector.tensor_copy(t16[:], t32[:])
                    nc.gpsimd.dma_start(
                        wq16[i * 256 : (i + 1) * 256, :].rearrange(
                            "(a p) h -> p a h", p=128
                        ),
                        t16[:],
                    )
                wd16 = dram.tile([DPC, HID], BF16, tag="wd16")
                t32 = wp.tile([128, 2, HID], F32, tag="w32")
                t16 = wp.tile([128, 2, HID], BF16, tag="w16")
                nc.gpsimd.dma_start(t32[:], wd[:].rearrange("(a p) h -> p a h", p=128))
                nc.vector.tensor_copy(t16[:], t32[:])
                nc.gpsimd.dma_start(
                    wd16[:].rearrange("(a p) h -> p a h", p=128), t16[:]
                )
                for kc in range(KC):
                    nc.scalar.dma_start_transpose(
                        wqT[:, kc, :], wq16[:, kc * 128 : (kc + 1) * 128]
                    )
                    nc.scalar.dma_start_transpose(
                        wdT[:, kc, :], wd16[:, kc * 128 : (kc + 1) * 128]
                    )

            # ---- phase 1: cast own hidden slice, AllGather bf16 hidden,
            #      transpose-load panels, QKV projection ----
            with (
                tc.tile_pool(name="hload", bufs=3) as hl,
                tc.tile_pool(name="hidT", bufs=2) as hp,
                tc.tile_pool(name="qkvps", bufs=2, space="PSUM") as qkps,
                tc.tile_pool(name="vps", bufs=2, space="PSUM") as vps,
            ):
                for tt in range(TPC // 128):
                    h32 = hl.tile([128, HID], F32, tag="h32")
                    ht = hl.tile([128, HID], BF16, tag="ht")
                    nc.sync.dma_start(h32[:], hidden[tt * 128 : (tt + 1) * 128, :])
                    if tt % 2 == 0:
                        nc.vector.tensor_copy(ht[:], h32[:])
                    else:
                        nc.scalar.copy(ht[:], h32[:])
                    nc.gpsimd.dma_start(h16own[tt * 128 : (tt + 1) * 128, :], ht[:])
                all_gather(h16own, h16all)

                for p in range(NPANEL):
                    hidT = hp.tile([128, KC, PTOK], BF16, tag="hidT")
                    for kc in range(KC):
                        nc.scalar.dma_start_transpose(
                            hidT[:, kc, :],
                            h16all[p * PTOK : (p + 1) * PTOK, kc * 128 : (kc + 1) * 128],
                        )
                    # q/k rows (T layout): row-blocks m: 0=q.h0 1=k.h0 2=v.h0 3=q.h1 ...
                    for m in (0, 1, 3, 4):
                        h = m // 3
                        dst = qT if (m % 3 == 0) else kT
                        ps = qkps.tile([128, PTOK], F32, tag="qk")
                        nc.tensor.matmul(
                            ps[:],
                            bias_sb[:, m * 128 : (m + 1) * 128],
                            ones_bf[:, :PTOK],
                            start=True,
                            stop=False,
                        )
                        for kc in range(KC):
                            nc.tensor.matmul(
                                ps[:],
                                wqT[:, kc, m * 128 : (m + 1) * 128],
                                hidT[:, kc, :],
                                start=False,
                                stop=(kc == KC - 1),
                            )
                        nc.scalar.copy(
                            dst[:, h, :, p * PS : (p + 1) * PS],
                            ps.rearrange("p (s b) -> p b s", b=BATCH),
                        )
                    # v rows (natural layout [t, (h dv)]), per batch, consecutive s
                    vrows = wqT[:, :, :].rearrange(
                        "p k (h three d) -> p k three h d", three=3, d=128
                    )
                    brows = bias_sb.rearrange(
                        "o (h three d) -> o three h d", three=3, d=128
                    )
                    for b in range(BATCH):
                        for tb in range(PS // 128):
                            tc_idx = p * (PS // 128) + tb
                            ps = vps.tile([128, HPC * HD], F32, tag="v")
                            nc.tensor.matmul(
                                ps[:],
                                ones_bf[:, :128],
                                brows[:, 2, :, :],
                                start=True,
                                stop=False,
                            )
                            for kc in range(KC):
                                lhs = hidT[:, kc, :].rearrange(
                                    "p (s b) -> p b s", b=BATCH
                                )[:, b, tb * 128 : (tb + 1) * 128]
                                nc.tensor.matmul(
                                    ps[:],
                                    lhs,
                                    vrows[:, kc, 2, :, :],
                                    start=False,
                                    stop=(kc == KC - 1),
                                )
                            nc.vector.tensor_copy(
                                vst[:, b, :, tc_idx, :],
                                ps.rearrange("p (h d) -> p h d", h=HPC),
                            )

            # ---- phase 2: causal attention per (b, head, s-block) ----
            with (
                tc.tile_pool(name="attn", bufs=4) as ap,
                tc.tile_pool(name="acc", bufs=2) as accp,
                tc.tile_pool(name="sps", bufs=2, space="PSUM") as sps,
                tc.tile_pool(name="cps", bufs=2, space="PSUM") as cps,
                tc.tile_pool(name="sumps", bufs=1, space="PSUM") as sumps,
                tc.tile_pool(name="bps", bufs=1, space="PSUM") as bps,
            ):
                def attend(b, h, sb):
                    s0 = sb * SB
                    q_rhs = qT[:, h, b, s0 : s0 + SB]
                    nchunk = (sb + 1) * (SB // 128)
                    ctx_ps = cps.tile([128, SB], F32, tag="ctx", name="ctx")
                    acc = accp.tile([128, SB], F32, tag="acc", name="acc")
                    for kc in range(nchunk):
                        s_ps = sps.tile([128, SB], F32, tag="s", name="s")
                        nc.tensor.matmul(
                            s_ps[:],
                            kT[:, h, b, kc * 128 : (kc + 1) * 128],
                            q_rhs,
                            start=True,
                            stop=True,
                        )
                        e = ap.tile([128, SB], BF16, tag="e", name="e")
                        nc.scalar.activation(
                            e[:],
                            s_ps[:],
                            mybir.ActivationFunctionType.Exp,
                            scale=SCALE,
                        )
                        off = kc * 128 - s0
                        if off >= 0:
                            # diagonal tile: keep where s >= t, else 0
                            nc.gpsimd.affine_select(
                                out=e[:],
                                in_=e[:],
                                compare_op=mybir.AluOpType.is_ge,
                                fill=0.0,
                                base=-off,
                                pattern=[[1, SB]],
                                channel_multiplier=-1,
                            )
                        if kc == 0:
                            nc.vector.tensor_copy(acc[:], e[:])
                        else:
                            nc.vector.tensor_add(acc[:], acc[:], e[:])
                        nc.tensor.matmul(
                            ctx_ps[:],
                            vst[:, b, h, kc, :],
                            e[:],
                            start=(kc == 0),
                            stop=(kc == nchunk - 1),
                        )
                    sum_ps = sumps.tile([1, SB], F32, tag="sum", name="sum")
                    nc.tensor.matmul(
                        sum_ps[:], ones_col[:], acc[:], start=True, stop=True
                    )
                    recip = ap.tile([1, SB], F32, tag="recip", name="recip")
                    nc.vector.reciprocal(recip[:], sum_ps[:])
                    b_ps = bps.tile([128, SB], F32, tag="bcast", name="bcast")
                    nc.tensor.matmul(
                        b_ps[:], ones_row[:], recip[:], start=True, stop=True
                    )
                    bcast = ap.tile([128, SB], F32, tag="bcsb", name="bcsb")
                    nc.scalar.copy(bcast[:], b_ps[:])
                    ctx_sb = ap.tile([128, SB], BF16, tag="ctxsb", name="ctxsb")
                    nc.vector.tensor_mul(ctx_sb[:], ctx_ps[:], bcast[:])
                    ag = ag_in_a if s0 < HSEQ else ag_in_b
                    nc.gpsimd.dma_start(
                        ag[h * 128 : (h + 1) * 128, b, s0 % HSEQ : s0 % HSEQ + SB],
                        ctx_sb[:],
                    )

                # s-block-major order so each AG half can fire early
                for sb in range(NSB // 2):
                    for b in range(BATCH):
                        for h in range(HPC):
                            attend(b, h, sb)
                all_gather(ag_in_a, ag_out_a)
                for sb in range(NSB // 2, NSB):
                    for b in range(BATCH):
                        for h in range(HPC):
                            attend(b, h, sb)
                all_gather(ag_in_b, ag_out_b)

            # ---- phase 3: dense (out-dim sharded), first half overlaps AG_b ----
            with (
                tc.tile_pool(name="gt", bufs=3) as gp,
                tc.tile_pool(name="osb", bufs=2) as op,
                tc.tile_pool(name="dps", bufs=8, space="PSUM") as dps,
            ):
                for sb in range(NSB):
                    ag_out_x = ag_out_a if sb < NSB // 2 else ag_out_b
                    s0 = sb * SB
                    sh = s0 % HSEQ
                    for b in range(BATCH):
                        psums = []
                        for _pi in range(SB // 128):
                            dpt = dps.tile([128, DPC], F32, tag="d", name=f"d{_pi}")
                            psums.append(dpt)
                        gt = gp.tile([128, KC, SB], BF16, tag="gt", name="gt")
                        nc.sync.dma_start(
                            gt[:],
                            ag_out_x[:, b, sh : sh + SB].rearrange(
                                "(k p) s -> p k s", p=128
                            ),
                        )
                        for kc in range(KC):
                            for sub in range(SB // 128):
                                nc.tensor.matmul(
                                    psums[sub][:],
                                    gt[:, kc, sub * 128 : (sub + 1) * 128],
                                    wdT[:, kc, :],
                                    start=(kc == 0),
                                    stop=(kc == KC - 1),
                                )
                        o = op.tile([128, SB // 128, DPC], F32, tag="o", name="o")
                        for sub in range(SB // 128):
                            nc.scalar.copy(o[:, sub, :], psums[sub][:])
                        nc.sync.dma_start(
                            out[:]
                            .rearrange("(s b) d -> s b d", b=BATCH)[
                                s0 : s0 + SB, b, :
                            ]
                            .rearrange("(sub p) d -> p sub d", p=128),
                            o[:],
                        )
    nc.finalize()
    return nc


def _get_nc():
    global _compiled
    if _compiled is None:
        _compiled = _build()
    return _compiled


LAST_RESULTS = {}


def kernel(hidden_states, attention_mask, qkv_weight, qkv_bias, dense_weight, dense_bias):
    nc = _get_nc()
    in_maps = _make_in_maps(hidden_states, qkv_weight, qkv_bias, dense_weight)
    res = run_bass_kernel_spmd(nc, in_maps, core_ids=list(range(NCORES)), trace=False)
    LAST_RESULTS["exec_time_ns"] = res.exec_time_ns
    LAST_RESULTS["res"] = res
    outs = [np.asarray(res.results[c]["out"]) for c in range(NCORES)]
    full = np.concatenate(outs, axis=1).reshape(SEQ, BATCH, HID)
    return full, np.asarray(dense_bias)


def _make_in_maps(hidden_states, qkv_weight, qkv_bias, dense_weight):
    hs = np.asarray(hidden_states, np.float32).reshape(TOK, HID)
    wq = np.asarray(qkv_weight, np.float32)
    bq = np.asarray(qkv_bias, np.float32)
    wde = np.asarray(dense_weight, np.float32)
    in_maps = []
    for c in range(NCORES):
        in_maps.append(
            {
                "hidden": np.ascontiguousarray(hs[c * TPC : (c + 1) * TPC]),
                "wqkv": np.ascontiguousarray(wq[c * QROWS : (c + 1) * QROWS]),
                "bqkv": np.ascontiguousarray(bq[c * QROWS : (c + 1) * QROWS]).reshape(1, QROWS),
                "wd": np.ascontiguousarray(wde[c * DPC : (c + 1) * DPC]),
            }
        )
    return in_maps


def bench_kernel(inputs, iters=8):
    """Steady-state timing: compile the sharded callable once, run `iters`
    times with device-resident inputs, return list of per-call wall seconds."""
    import time

    import jax
    from jax.experimental.shard_map import shard_map
    from jax.sharding import Mesh, PartitionSpec

    from concourse import bass2jax, mybir as _mybir

    nc = _get_nc()
    in_maps = _make_in_maps(
        inputs["hidden_states"], inputs["qkv_weight"], inputs["qkv_bias"],
        inputs["dense_weight"],
    )
    bass2jax.install_neuronx_cc_hook()

    partition_name = nc.partition_id_tensor.name if nc.partition_id_tensor else None
    in_names, out_names, out_avals, zero_outs = [], [], [], []
    for alloc in nc.m.functions[0].allocations:
        if not isinstance(alloc, _mybir.MemoryLocationSet):
            continue
        name = alloc.memorylocations[0].name
        if alloc.kind == "ExternalInput":
            if name != partition_name:
                in_names.append(name)
        elif alloc.kind == "ExternalOutput":
            out_names.append(name)
            np_dtype = _mybir.dt.np(alloc.dtype)
            out_avals.append(jax.core.ShapedArray(tuple(alloc.tensor_shape), np_dtype))
            zero_outs.append(np.zeros(tuple(alloc.tensor_shape), np_dtype))
    n_params = len(in_names)
    all_names = in_names + out_names
    if partition_name is not None:
        all_names = all_names + [partition_name]

    def _body(*args):
        operands = list(args)
        if partition_name is not None:
            operands.append(bass2jax.partition_id_tensor())
        outs = bass2jax._bass_exec_p.bind(
            *operands,
            out_avals=tuple(out_avals),
            in_names=tuple(all_names),
            out_names=tuple(out_names),
            lowering_input_output_aliases=(),
            sim_require_finite=True,
            sim_require_nnan=True,
            nc=nc,
        )
        return tuple(outs)

    devices = jax.devices()[:NCORES]
    mesh = Mesh(np.asarray(devices), ("core",))
    nin = n_params + len(out_names)
    sharded = jax.jit(
        shard_map(
            _body,
            mesh=mesh,
            in_specs=(PartitionSpec("core"),) * nin,
            out_specs=(PartitionSpec("core"),) * len(out_names),
            check_rep=False,
        ),
        keep_unused=True,
    )
    concat_in = [
        np.concatenate([np.asarray(in_maps[c][nm]) for c in range(NCORES)], axis=0)
        for nm in in_names
    ]
    concat_zero = [np.zeros((NCORES * z.shape[0], *z.shape[1:]), z.dtype) for z in zero_outs]
    args = [jax.device_put(a) for a in concat_in + concat_zero]
    out = sharded(*args)  # compile + warmup
    jax.block_until_ready(out)
    times = []
    for _ in range(iters):
        t0 = time.perf_counter()
        out = sharded(*args)
        jax.block_until_ready(out)
        times.append(time.perf_counter() - t0)
    return times, out, out_names
